# revision 22
# baseline (speedup 1.0000x reference)
"""Trainium2 Bass kernel for nn_AttentionBlock (biased dense attention).

Math:  x' = x + phi_degree + phi_3d_sum
       S  = (x' Wq)(x' Wk)^T * scaling + phi_spd + phi_edge + phi_3d
       out = softmax(S, axis=-1) @ (x' Wv)

Strategy (8 cores, sequence parallel on q). Host prep is layout-only plus
small [n,d] matmuls: B = ASCALE * x' (scaling Wq Wk^T) (the folded query
projection), V = x' Wv (+ ones column for softmax denominators, fp16), the
transposed phi bias sum quantized to int16 at ASCALE, and x'^T in device
layout (fp32).

The S^T matmuls run in float32r: TRN2's fast fp32 path that ingests
operands rounded to 11 explicit mantissa bits at 1 PE cycle/row (4x the
plain-fp32 rate, same rate as bf16) when the matmul destination is >= 256
wide and even (s3d3_mm_fp32r ISA restriction).

Device-side, per core (1/8 of the n^2 work):
  - S^T[k, q] psum tiles = xt-block.T @ B^T-chunk (2 f32r matmuls, d=256
    contraction). Computing S TRANSPOSED makes exp(S^T) directly usable as
    the stationary operand of the P@V matmul - no on-chip transposes.
  - the phi bias arrives as int16 (absolute quantization error 2^-12 on the
    logits - 8x better at the tails than fp16 and half the bytes of fp32)
    and is added into the S psum by one DVE tensor_add per tile (int16 ->
    fp32 conversion is exact on DVE; GPSIMD cannot write PSUM).
  - exp((S/ASCALE) - 18) on ScalarE writes fp16 directly (the e^-18
    cancels in softmax normalization and keeps max P = e^(S_max-18) in
    fp16 range; S_max ~ 22 for these inputs).
  - P@V accumulates over all k blocks in fp16 (P and V quantization are
    each ~2^-11 relative - same error class as the f32r logits path,
    measured identical end-to-end). Softmax denominators come free from
    the ones-column appended to V. PV issue lags S by `pvlag` k-blocks
    (software pipelining) so TensorE never waits on the DVE+ScalarE chain.

Measured on 8xTRN2 (reps-in-NEFF differencing): ~146 us vs the 479 us
baseline, rel err 6.3e-3 against the fp32 reference (gate: 2e-2).

kernel(**inputs) -> full [8192, 256] fp32 output.
"""

import contextlib

import numpy as np

import concourse.bacc as bacc
import concourse.tile as tile
from concourse import mybir
from concourse.bass_utils import run_bass_kernel_spmd

N_FULL = 8192
D = 256
CORES = 8
SCALING = 0.0625

f32 = mybir.dt.float32
f32r = mybir.dt.float32r
f16 = mybir.dt.float16
i16 = mybir.dt.int16

# Logits are computed at ASCALE scale so the phi bias sum can ship as int16
# (max |phi|*ASCALE ~ 20.4k < 32767). exp divides back via its affine scale.
ASCALE = 2048.0

# P@V runs in fp16 (P quantization ~2^-11 relative, V likewise) -- measured
# end-to-end error is the same as the f32r path, and V ships at half the
# bytes. exp uses bias -18 so max P = e^(22.1-18) stays in fp16 range.
PV_F16 = True


def build_attention_nc(
    n,
    n_loc,
    d=D,
    cores=CORES,
    reps=1,
    hoist_inputs=False,
    parts=None,
    pv_vw=None,
    pv_f16=PV_F16,
    psq_bufs=4,
    pvlag=4,
    add_sbuf=False,
):
    """Build the SPMD Bass program (one program, runs on all cores)."""
    parts = parts if parts is not None else {"phi", "add", "exp", "pv", "epi"}
    assert n % 1024 == 0 and n_loc % 512 == 0 and d == 256
    QCH = 512  # q-chunk: free dim of S^T tiles (one PSUM bank)
    n_qc = n_loc // QCH
    n_kb = n // 128  # k blocks
    n_db = d // 128  # 2
    KSLAB = 8  # k-blocks per phi DMA slab (host layout fixes this at 8)
    n_sb = n_kb // KSLAB
    vw = pv_vw if pv_vw is not None else d + 2  # ones col; even width (fp32r)
    PVLAG = pvlag  # PV matmuls lag S by this many k-blocks

    nc = bacc.Bacc("TRN2", target_bir_lowering=False, debug=False, num_devices=cores)

    def param(name, shape, dt=f32):
        return nc.declare_dram_parameter(name, shape, dt, isOutput=False)

    xpt = param("xpt", [128, n_db, n], f32r)  # x'^T device layout
    btq = param("btq", [128, n_db, n_loc], f32r)  # B_loc^T (ASCALE-scaled)
    pv_t = f16 if pv_f16 else f32r
    vv = param("vv", [128, n_kb, vw], pv_t)  # V rows blocked + ones col
    phi = param("phi_i16", [n_qc, n_sb, 128, KSLAB, QCH], i16)
    out = nc.declare_dram_parameter("out", [n_loc, d], f32, isOutput=True)

    with tile.TileContext(nc) as tc:
        with (
            tc.tile_pool(name="res", bufs=1) as res,
        ):

            def load_inputs():
                bias12 = res.tile([128, 1], f32, name="bias12")
                nc.vector.memset(bias12, -18.0 if pv_f16 else -12.0)
                # small operand first so the first S matmuls can start early
                bt_t = res.tile([128, n_db, n_loc], f32r, name="bt_t")
                nc.gpsimd.dma_start(out=bt_t, in_=btq[:, :, :])
                xt_t = res.tile([128, n_db, n], f32r, name="xt_t")
                v_t = res.tile([128, n_kb, vw], pv_t, name="v_t")
                XCH = 2048
                VCH = 16
                for c in range(n // XCH):
                    k0 = c * XCH
                    nc.gpsimd.dma_start(
                        out=xt_t[:, :, k0 : k0 + XCH], in_=xpt[:, :, k0 : k0 + XCH]
                    )
                    kb0 = c * VCH
                    nc.gpsimd.dma_start(
                        out=v_t[:, kb0 : kb0 + VCH, :], in_=vv[:, kb0 : kb0 + VCH, :]
                    )
                return bias12, bt_t, xt_t, v_t

            if hoist_inputs:
                bias12, bt_t, xt_t, v_t = load_inputs()
            loop_ctx = tc.For_i(0, reps, 1) if reps > 1 else contextlib.nullcontext()
            with loop_ctx:
                if not hoist_inputs:
                    bias12, bt_t, xt_t, v_t = load_inputs()
                with (
                    tc.tile_pool(name="phC", bufs=3) as phC,
                    tc.tile_pool(name="ptC", bufs=PVLAG + 3) as ptC,
                    tc.tile_pool(name="obC", bufs=3) as obC,
                    tc.tile_pool(name="sxC", bufs=3) as sxC,
                    tc.tile_pool(name="psS", bufs=psq_bufs, space="PSUM") as psS,
                    tc.tile_pool(name="psO", bufs=1, space="PSUM") as psO,
                ):
                    # phi slab prefetch across the flattened (qc, sb) sequence
                    slab_tiles = {}

                    def load_slab(g):
                        if g >= n_qc * n_sb or "phi" not in parts:
                            return
                        qc_, sb_ = divmod(g, n_sb)
                        t_ = phC.tile(
                            [128, KSLAB, QCH], i16, tag="phi", name=f"phi{g}"
                        )
                        nc.sync.dma_start(out=t_, in_=phi[qc_, sb_])
                        slab_tiles[g] = t_

                    load_slab(0)
                    load_slab(1)

                    for qc in range(n_qc):
                        out_ps = [
                            psO.tile(
                                [128, vw], f32, tag=f"out{t}", name=f"outp{qc}_{t}"
                            )
                            for t in range(QCH // 128)
                        ]

                        def pv(kbo, pto):
                            for t in range(QCH // 128):
                                nc.tensor.matmul(
                                    out_ps[t],
                                    pto[:, t * 128 : (t + 1) * 128],
                                    v_t[:, kbo, :],
                                    start=(kbo == 0),
                                    stop=(kbo == n_kb - 1),
                                )

                        pend = []
                        for kb in range(n_kb):
                            sb = kb // KSLAB
                            g = qc * n_sb + sb
                            if kb % KSLAB == 0:
                                load_slab(g + 2)
                            s_ps = psS.tile([128, QCH], f32, tag="s")
                            for db in range(n_db):
                                nc.tensor.matmul(
                                    s_ps,
                                    xt_t[:, db, kb * 128 : (kb + 1) * 128],
                                    bt_t[:, db, qc * QCH : (qc + 1) * QCH],
                                    start=(db == 0),
                                    stop=(db == n_db - 1),
                                )
                            exp_in = s_ps
                            if "add" in parts:
                                if add_sbuf:
                                    sx = sxC.tile([128, QCH], f32, tag="sx")
                                    nc.vector.tensor_add(
                                        sx, s_ps, slab_tiles[g][:, kb % KSLAB, :]
                                    )
                                    exp_in = sx
                                else:
                                    nc.vector.tensor_add(
                                        s_ps, s_ps, slab_tiles[g][:, kb % KSLAB, :]
                                    )
                            if "exp" in parts:
                                pt = ptC.tile([128, QCH], pv_t, tag="pt")
                                nc.scalar.activation(
                                    out=pt,
                                    in_=exp_in,
                                    func=mybir.ActivationFunctionType.Exp,
                                    bias=bias12,
                                    scale=1.0 / ASCALE,
                                )
                                if "pv" in parts:
                                    pend.append((kb, pt))
                                    if len(pend) > PVLAG:
                                        pv(*pend.pop(0))
                        for kbo, pto in pend:
                            pv(kbo, pto)

                        if "epi" in parts and "pv" in parts:
                            for t in range(QCH // 128):
                                rs = obC.tile([128, 1], f32, tag="rs")
                                dc = min(d, vw - 2)
                                nc.vector.reciprocal(rs, out_ps[t][:, dc : dc + 1])
                                ob = obC.tile([128, d], f32, tag="ob")
                                nc.vector.tensor_scalar_mul(ob, out_ps[t][:, :d], rs)
                                r0 = qc * QCH + t * 128
                                nc.sync.dma_start(out=out[r0 : r0 + 128, :], in_=ob)
    nc.compile()
    return nc


def _dev_rows(a, vw=None):
    """[m, d] -> [128, m//128, d(+1 ones)] row-blocked device layout."""
    m, d = a.shape
    arr = a.reshape(m // 128, 128, d).transpose(1, 0, 2)
    if vw is not None:
        ones = np.ones((128, m // 128, vw - d), np.float32)
        arr = np.concatenate([arr, ones], axis=2)
    return np.ascontiguousarray(arr)


def _dev_cols(aT):
    """[d, m] -> [128, d//128, m] device layout (partition-major)."""
    d, m = aT.shape
    return np.ascontiguousarray(aT.reshape(d // 128, 128, m).transpose(1, 0, 2))


def _make_in_maps(
    xp, A, Wv, phi_spd, phi_edge, phi_3d, n_loc, cores=CORES, pv_f16=False
):
    n = xp.shape[0]
    d = xp.shape[1]
    QCH = 512
    KSLAB = 8
    n_qc = n_loc // QCH
    n_sb = n // (128 * KSLAB)

    xpt = _dev_cols(np.ascontiguousarray(xp.T))
    B = (xp.astype(np.float64) @ A.astype(np.float64) * ASCALE).astype(np.float32)
    V = (xp.astype(np.float64) @ Wv.astype(np.float64)).astype(np.float32)
    vv = _dev_rows(V, vw=d + 2)
    if pv_f16:
        vv = vv.astype(np.float16)

    phisum = phi_spd + phi_edge
    phisum += phi_3d
    phisum *= np.float32(ASCALE)
    phi_i16 = np.clip(np.rint(phisum), -32767, 32767).astype(np.int16)
    del phisum

    in_maps = []
    for c in range(cores):
        r0, r1 = c * n_loc, (c + 1) * n_loc
        bt = _dev_cols(np.ascontiguousarray(B[r0:r1].T))
        ph = np.ascontiguousarray(
            phi_i16[r0:r1]
            .T.reshape(n_sb, KSLAB, 128, n_qc, QCH)
            .transpose(3, 0, 2, 1, 4)
        )
        in_maps.append(
            {
                "xpt": xpt,
                "btq": bt,
                "vv": vv,
                "phi_i16": ph,
            }
        )
    return in_maps


_CACHED_NC = {}


def _get_nc(n, n_loc):
    key = (n, n_loc)
    if key not in _CACHED_NC:
        _CACHED_NC[key] = build_attention_nc(n, n_loc)
    return _CACHED_NC[key]


def kernel(x, phi_degree, phi_3d_sum, phi_3d, phi_spd, phi_edge, Wq, Wk, Wv):
    x = np.asarray(x, dtype=np.float32)
    phi_degree = np.asarray(phi_degree, dtype=np.float32)
    phi_3d_sum = np.asarray(phi_3d_sum, dtype=np.float32)
    phi_3d = np.asarray(phi_3d, dtype=np.float32)
    phi_spd = np.asarray(phi_spd, dtype=np.float32)
    phi_edge = np.asarray(phi_edge, dtype=np.float32)
    Wq = np.asarray(Wq, dtype=np.float32)
    Wk = np.asarray(Wk, dtype=np.float32)
    Wv = np.asarray(Wv, dtype=np.float32)

    n = x.shape[0]
    n_loc = n // CORES
    xp = x + phi_degree + phi_3d_sum
    A = (SCALING * (Wq.astype(np.float64) @ Wk.astype(np.float64).T)).astype(
        np.float32
    )

    nc = _get_nc(n, n_loc)
    in_maps = _make_in_maps(
        xp, A, Wv, phi_spd, phi_edge, phi_3d, n_loc, pv_f16=PV_F16
    )
    res = run_bass_kernel_spmd(nc, in_maps, list(range(CORES)))
    return np.concatenate([res.results[c]["out"] for c in range(CORES)], axis=0)


# revision 24
# speedup vs baseline: 1.2967x; 1.2967x over previous
"""Trainium2 Bass kernel for nn_AttentionBlock (biased dense attention).

Math:  x' = x + phi_degree + phi_3d_sum
       S  = (x' Wq)(x' Wk)^T * scaling + phi_spd + phi_edge + phi_3d
       out = softmax(S, axis=-1) @ (x' Wv)

Strategy (8 cores, sequence parallel on q). Host prep is layout-only plus
small [n,d] matmuls: B = ASCALE * x' (scaling Wq Wk^T) (the folded query
projection), V = x' Wv (+ ones column for softmax denominators, fp16), the
transposed phi bias sum quantized to int16 at ASCALE, and x'^T in device
layout (fp32).

The S^T matmuls run in float32r: TRN2's fast fp32 path that ingests
operands rounded to 11 explicit mantissa bits at 1 PE cycle/row (4x the
plain-fp32 rate, same rate as bf16) when the matmul destination is >= 256
wide and even (s3d3_mm_fp32r ISA restriction).

Device-side, per core (1/8 of the n^2 work):
  - S^T[k, q] psum tiles = xt-block.T @ B^T-chunk (2 f32r matmuls, d=256
    contraction). Computing S TRANSPOSED makes exp(S^T) directly usable as
    the stationary operand of the P@V matmul - no on-chip transposes.
  - the phi bias arrives as int16 (absolute quantization error 2^-12 on the
    logits - 8x better at the tails than fp16 and half the bytes of fp32)
    and is added into the S psum by one DVE tensor_add per tile (int16 ->
    fp32 conversion is exact on DVE; GPSIMD cannot write PSUM).
  - exp((S/ASCALE) - 18) on ScalarE writes fp16 directly (the e^-18
    cancels in softmax normalization and keeps max P = e^(S_max-18) in
    fp16 range; S_max ~ 22 for these inputs).
  - P@V accumulates over all k blocks in fp16 (P and V quantization are
    each ~2^-11 relative - same error class as the f32r logits path,
    measured identical end-to-end). Softmax denominators come free from
    the ones-column appended to V. PV issue lags S by `pvlag` k-blocks
    (software pipelining) so TensorE never waits on the DVE+ScalarE chain.

Measured on 8xTRN2 (reps-in-NEFF differencing): ~146 us vs the 479 us
baseline, rel err 6.3e-3 against the fp32 reference (gate: 2e-2).

kernel(**inputs) -> full [8192, 256] fp32 output.
"""

import contextlib

import numpy as np

import concourse.bacc as bacc
import concourse.tile as tile
from concourse import mybir
from concourse.bass_utils import run_bass_kernel_spmd

N_FULL = 8192
D = 256
CORES = 8
SCALING = 0.0625

f32 = mybir.dt.float32
f32r = mybir.dt.float32r
f16 = mybir.dt.float16
i16 = mybir.dt.int16

# Logits are computed at ASCALE scale so the phi bias sum can ship as int16
# (max |phi|*ASCALE ~ 20.4k < 32767). exp divides back via its affine scale.
ASCALE = 2048.0

# P@V runs in fp16 (P quantization ~2^-11 relative, V likewise) -- measured
# end-to-end error is the same as the f32r path, and V ships at half the
# bytes. exp uses bias -18 so max P = e^(22.1-18) stays in fp16 range.
PV_F16 = True


def build_attention_nc(
    n,
    n_loc,
    d=D,
    cores=CORES,
    reps=1,
    hoist_inputs=False,
    parts=None,
    pv_vw=None,
    pv_f16=PV_F16,
    psq_bufs=4,
    pvlag=4,
    add_sbuf=False,
):
    """Build the SPMD Bass program (one program, runs on all cores)."""
    parts = parts if parts is not None else {"phi", "add", "exp", "pv", "epi"}
    assert n % 1024 == 0 and n_loc % 512 == 0 and d == 256
    QCH = 512  # q-chunk: free dim of S^T tiles (one PSUM bank)
    n_qc = n_loc // QCH
    n_kb = n // 128  # k blocks
    n_db = d // 128  # 2
    KSLAB = 8  # k-blocks per phi DMA slab (host layout fixes this at 8)
    n_sb = n_kb // KSLAB
    vw = pv_vw if pv_vw is not None else d + 2  # ones col; even width (fp32r)
    PVLAG = pvlag  # PV matmuls lag S by this many k-blocks

    nc = bacc.Bacc("TRN2", target_bir_lowering=False, debug=False, num_devices=cores)

    def param(name, shape, dt=f32):
        return nc.declare_dram_parameter(name, shape, dt, isOutput=False)

    xpt = param("xpt", [128, n_db, n], f32r)  # x'^T device layout
    btq = param("btq", [128, n_db, n_loc], f32r)  # B_loc^T (ASCALE-scaled)
    pv_t = f16 if pv_f16 else f32r
    vv = param("vv", [128, n_kb, vw], pv_t)  # V rows blocked + ones col
    phi = param("phi_i16", [n_qc, n_sb, 128, KSLAB, QCH], i16)
    out = nc.declare_dram_parameter("out", [n_loc, d], f32, isOutput=True)

    with tile.TileContext(nc) as tc:
        with (
            tc.tile_pool(name="res", bufs=1) as res,
        ):

            def load_inputs():
                bias12 = res.tile([128, 1], f32, name="bias12")
                nc.vector.memset(bias12, -18.0 if pv_f16 else -12.0)
                # small operand first so the first S matmuls can start early
                bt_t = res.tile([128, n_db, n_loc], f32r, name="bt_t")
                nc.gpsimd.dma_start(out=bt_t, in_=btq[:, :, :])
                xt_t = res.tile([128, n_db, n], f32r, name="xt_t")
                v_t = res.tile([128, n_kb, vw], pv_t, name="v_t")
                xsplit = [0, 512, 2048, 4096, 6144, n]
                vsplit = [0, 4, 16, 32, 48, n_kb]
                for c in range(len(xsplit) - 1):
                    k0, k1 = xsplit[c], xsplit[c + 1]
                    nc.gpsimd.dma_start(
                        out=xt_t[:, :, k0:k1], in_=xpt[:, :, k0:k1]
                    )
                    b0, b1 = vsplit[c], vsplit[c + 1]
                    nc.gpsimd.dma_start(
                        out=v_t[:, b0:b1, :], in_=vv[:, b0:b1, :]
                    )
                return bias12, bt_t, xt_t, v_t

            if hoist_inputs:
                bias12, bt_t, xt_t, v_t = load_inputs()
            loop_ctx = tc.For_i(0, reps, 1) if reps > 1 else contextlib.nullcontext()
            with loop_ctx:
                if not hoist_inputs:
                    bias12, bt_t, xt_t, v_t = load_inputs()
                with (
                    tc.tile_pool(name="phC", bufs=3) as phC,
                    tc.tile_pool(name="ptC", bufs=PVLAG + 3) as ptC,
                    tc.tile_pool(name="obC", bufs=3) as obC,
                    tc.tile_pool(name="sxC", bufs=3) as sxC,
                    tc.tile_pool(name="psS", bufs=max(2, psq_bufs // 2), space="PSUM") as psS,
                    tc.tile_pool(name="psO", bufs=1, space="PSUM") as psO,
                ):
                    # phi slab prefetch across the flattened (qc, sb) sequence
                    slab_tiles = {}

                    def load_slab(g):
                        if g >= n_qc * n_sb or "phi" not in parts:
                            return
                        qc_, sb_ = divmod(g, n_sb)
                        t_ = phC.tile(
                            [128, KSLAB, QCH], i16, tag="phi", name=f"phi{g}"
                        )
                        nc.sync.dma_start(out=t_, in_=phi[qc_, sb_])
                        slab_tiles[g] = t_

                    load_slab(0)
                    load_slab(1)

                    for qc in range(n_qc):
                        out_ps = [
                            psO.tile(
                                [128, vw], f32, tag=f"out{t}", name=f"outp{qc}_{t}"
                            )
                            for t in range(QCH // 128)
                        ]

                        def pv(kbo, pto):
                            for t in range(QCH // 128):
                                nc.tensor.matmul(
                                    out_ps[t],
                                    pto[:, t * 128 : (t + 1) * 128],
                                    v_t[:, kbo, :],
                                    start=(kbo == 0),
                                    stop=(kbo == n_kb - 1),
                                )

                        def pv_pair(kp, pt2):
                            for j in range(2):
                                kbo = 2 * kp + j
                                for t in range(QCH // 128):
                                    nc.tensor.matmul(
                                        out_ps[t],
                                        pt2[:, j, t * 128 : (t + 1) * 128],
                                        v_t[:, kbo, :],
                                        start=(kbo == 0),
                                        stop=(kbo == n_kb - 1),
                                    )

                        pend = []
                        for kp in range(n_kb // 2):
                            kb0 = 2 * kp
                            sb = kb0 // KSLAB
                            g = qc * n_sb + sb
                            if kb0 % KSLAB == 0:
                                load_slab(g + 2)
                            pair = psS.tile([128, 2, QCH], f32, tag="s")
                            for j in range(2):
                                kb = kb0 + j
                                for db in range(n_db):
                                    nc.tensor.matmul(
                                        pair[:, j, :],
                                        xt_t[:, db, kb * 128 : (kb + 1) * 128],
                                        bt_t[:, db, qc * QCH : (qc + 1) * QCH],
                                        start=(db == 0),
                                        stop=(db == n_db - 1),
                                    )
                            if "add" in parts:
                                j0 = kb0 % KSLAB
                                nc.vector.tensor_add(
                                    pair, pair, slab_tiles[g][:, j0 : j0 + 2, :]
                                )
                            if "exp" in parts:
                                pt2 = ptC.tile([128, 2, QCH], pv_t, tag="pt")
                                nc.scalar.activation(
                                    out=pt2,
                                    in_=pair,
                                    func=mybir.ActivationFunctionType.Exp,
                                    bias=bias12,
                                    scale=1.0 / ASCALE,
                                )
                                if "pv" in parts:
                                    pend.append((kp, pt2))
                                    if len(pend) > PVLAG // 2:
                                        pv_pair(*pend.pop(0))
                        for kpo, pto in pend:
                            pv_pair(kpo, pto)

                        if "epi" in parts and "pv" in parts:
                            for t in range(QCH // 128):
                                rs = obC.tile([128, 1], f32, tag="rs")
                                dc = min(d, vw - 2)
                                nc.vector.reciprocal(rs, out_ps[t][:, dc : dc + 1])
                                ob = obC.tile([128, d], f32, tag="ob")
                                nc.vector.tensor_scalar_mul(ob, out_ps[t][:, :d], rs)
                                r0 = qc * QCH + t * 128
                                nc.scalar.dma_start(out=out[r0 : r0 + 128, :], in_=ob)
    nc.compile()
    return nc


def _dev_rows(a, vw=None):
    """[m, d] -> [128, m//128, d(+1 ones)] row-blocked device layout."""
    m, d = a.shape
    arr = a.reshape(m // 128, 128, d).transpose(1, 0, 2)
    if vw is not None:
        ones = np.ones((128, m // 128, vw - d), np.float32)
        arr = np.concatenate([arr, ones], axis=2)
    return np.ascontiguousarray(arr)


def _dev_cols(aT):
    """[d, m] -> [128, d//128, m] device layout (partition-major)."""
    d, m = aT.shape
    return np.ascontiguousarray(aT.reshape(d // 128, 128, m).transpose(1, 0, 2))


def _make_in_maps(
    xp, A, Wv, phi_spd, phi_edge, phi_3d, n_loc, cores=CORES, pv_f16=False
):
    n = xp.shape[0]
    d = xp.shape[1]
    QCH = 512
    KSLAB = 8
    n_qc = n_loc // QCH
    n_sb = n // (128 * KSLAB)

    xpt = _dev_cols(np.ascontiguousarray(xp.T))
    B = (xp.astype(np.float64) @ A.astype(np.float64) * ASCALE).astype(np.float32)
    V = (xp.astype(np.float64) @ Wv.astype(np.float64)).astype(np.float32)
    vv = _dev_rows(V, vw=d + 2)
    if pv_f16:
        vv = vv.astype(np.float16)

    phisum = phi_spd + phi_edge
    phisum += phi_3d
    phisum *= np.float32(ASCALE)
    phi_i16 = np.clip(np.rint(phisum), -32767, 32767).astype(np.int16)
    del phisum

    in_maps = []
    for c in range(cores):
        r0, r1 = c * n_loc, (c + 1) * n_loc
        bt = _dev_cols(np.ascontiguousarray(B[r0:r1].T))
        ph = np.ascontiguousarray(
            phi_i16[r0:r1]
            .T.reshape(n_sb, KSLAB, 128, n_qc, QCH)
            .transpose(3, 0, 2, 1, 4)
        )
        in_maps.append(
            {
                "xpt": xpt,
                "btq": bt,
                "vv": vv,
                "phi_i16": ph,
            }
        )
    return in_maps


_CACHED_NC = {}


def _get_nc(n, n_loc):
    key = (n, n_loc)
    if key not in _CACHED_NC:
        _CACHED_NC[key] = build_attention_nc(n, n_loc)
    return _CACHED_NC[key]


def kernel(x, phi_degree, phi_3d_sum, phi_3d, phi_spd, phi_edge, Wq, Wk, Wv):
    x = np.asarray(x, dtype=np.float32)
    phi_degree = np.asarray(phi_degree, dtype=np.float32)
    phi_3d_sum = np.asarray(phi_3d_sum, dtype=np.float32)
    phi_3d = np.asarray(phi_3d, dtype=np.float32)
    phi_spd = np.asarray(phi_spd, dtype=np.float32)
    phi_edge = np.asarray(phi_edge, dtype=np.float32)
    Wq = np.asarray(Wq, dtype=np.float32)
    Wk = np.asarray(Wk, dtype=np.float32)
    Wv = np.asarray(Wv, dtype=np.float32)

    n = x.shape[0]
    n_loc = n // CORES
    xp = x + phi_degree + phi_3d_sum
    A = (SCALING * (Wq.astype(np.float64) @ Wk.astype(np.float64).T)).astype(
        np.float32
    )

    nc = _get_nc(n, n_loc)
    in_maps = _make_in_maps(
        xp, A, Wv, phi_spd, phi_edge, phi_3d, n_loc, pv_f16=PV_F16
    )
    res = run_bass_kernel_spmd(nc, in_maps, list(range(CORES)))
    return np.concatenate([res.results[c]["out"] for c in range(CORES)], axis=0)
